# revision 7
# baseline (speedup 1.0000x reference)
"""HEALPix conservative-layer (segment_reduce) Bass kernel for TRN2.

Problem (hardcoded):
  x5: (2,2,4,12288,1,32) f32, x6: (2,2,4,49152,1,32), x7: (2,2,4,196608,1,32)
  out5 = x5 + gmean4(x6)
  out6 = x6 - rep4(gmean4(x6)) + gmean4(x7)
  out7 = x7 - rep4(gmean4(x7))
  out = concat([out5, out6, out7], axis=3)   # (2,2,4,258048,1,32)

Sharding: flatten (b,v,t) -> 16 slices; 8 cores x 2 slices each. Everything is
local to a core.

Layout: one slice (N, 32) is contiguous in DRAM and a parent's 4 children are
128 contiguous floats, so view each slice as (128 partitions, N*32/128) with
each partition a contiguous DRAM block. Parent group-reduction is then along
the free dim and the parent-sum tile S (in the same layout) aligns elementwise
with the next-coarser level's view of the same partition.

Schedule (build_nc2, the one kernel() uses): stage-interleaved streaming.
Each stage covers a matched span (4*ch6 of the x7 view + ch6 of the x6 view),
so input and output DMA traffic from both zoom levels coexists throughout the
run instead of phase-serial x7->x6->x5; the vector-engine latency of any one
chunk hides under the other stages' DMA. Loads issue on the sync HWDGE queue
and stores on the scalar HWDGE queue so next-stage loads never queue behind
dependency-stalled stores. Cost model: 370.9us/core = DMA floor (132MB at
360GB/s = 367.0us) + fixed TileContext preamble/teardown; zero DMA idle
in between. build_nc (phase-serial SWDGE baseline) kept for comparison:
385.4us.
"""

import numpy as np

try:
    import concourse.bass as bass
except ImportError:  # pragma: no cover - fallback for odd sys.path setups
    import sys

    sys.path.insert(0, "/opt/trn_rl_repo")
    import concourse.bass as bass

import concourse.mybir as mybir
import concourse.tile as tile
from concourse.bass_utils import run_bass_kernel_spmd
from concourse.mybir import AluOpType

F = 32
B, V, T = 2, 2, 4
N5, N6, N7 = 12 * 4**5, 12 * 4**6, 12 * 4**7
N_CORES = 8
SLICES = B * V * T  # 16
S_PER_CORE = SLICES // N_CORES  # 2
NOUT = N5 + N6 + N7

# floats per partition in the (128, .) view of one slice
FL5 = N5 * F // 128  # 3072
FL6 = N6 * F // 128  # 12288
FL7 = N7 * F // 128  # 49152
# streaming chunk sizes (floats per partition); must be multiples of 128
CH7 = 3072
CH6 = 2048

_DT = mybir.dt.float32


def _legalize_waits(nc):
    """Split multi-sem-wait instructions: walrus codegen packs at most one
    sync wait into a TPB instruction, so move excess waits onto NoOps inserted
    immediately before (same engine => same in-order semantics)."""
    import copy as _copy

    from bass_rust import SyncInfo

    tmpl = bass.Bass("TRN2").vector.nop().ins
    n = 0
    for fn in nc.m.functions:
        for blk in fn.blocks:
            out = []
            changed = False
            for inst in blk.instructions:
                si = inst.sync_info
                if (si is not None and len(si.on_wait) > 1
                        and all(w.wait_mode == "sem-ge-imm"
                                for w in si.on_wait)):
                    waits = list(si.on_wait)
                    for w in waits[:-1]:
                        nop = _copy.copy(tmpl)
                        nop.name = f"WN-{n}"
                        n += 1
                        nop.engine = inst.engine
                        nop.sync_info = SyncInfo(on_wait=[w], on_update=[])
                        out.append(nop)
                    inst.sync_info = SyncInfo(on_wait=[waits[-1]],
                                              on_update=list(si.on_update))
                    changed = True
                out.append(inst)
            if changed:
                blk.instructions = out
    return nc


def build_nc(s_per_core=S_PER_CORE, fl5=FL5, ch7=CH7, ch6=CH6, reps=1,
             hwdge=False, bufs7=3, bufso7=2):
    """Build the per-core Bass module. All sizes in per-partition floats."""
    fl6, fl7 = 4 * fl5, 16 * fl5
    assert fl6 % 128 == 0 and fl7 % 128 == 0
    assert fl7 % ch7 == 0 and ch7 % 128 == 0
    assert fl6 % ch6 == 0 and ch6 % 128 == 0
    n5, n6, n7 = fl5 * 128 // F, fl6 * 128 // F, fl7 * 128 // F

    nc = bass.Bass("TRN2", target_bir_lowering=False, debug=False,
                   enable_asserts=False)
    dma_in = (lambda *a: nc.sync.dma_start(*a)) if hwdge else \
        (lambda *a: nc.gpsimd.dma_start(*a))
    dma_out = (lambda *a: nc.scalar.dma_start(*a)) if hwdge else \
        (lambda *a: nc.gpsimd.dma_start(*a))
    x5 = nc.dram_tensor("x5", [s_per_core, n5, F], _DT, kind="ExternalInput")
    x6 = nc.dram_tensor("x6", [s_per_core, n6, F], _DT, kind="ExternalInput")
    x7 = nc.dram_tensor("x7", [s_per_core, n7, F], _DT, kind="ExternalInput")
    out = nc.dram_tensor("out", [s_per_core, n5 + n6 + n7, F], _DT,
                         kind="ExternalOutput")

    with tile.TileContext(nc) as tc, \
            tc.tile_pool(name="in7", bufs=bufs7) as p7, \
            tc.tile_pool(name="out7", bufs=bufso7) as o7, \
            tc.tile_pool(name="in6", bufs=2) as p6, \
            tc.tile_pool(name="out6", bufs=2) as o6, \
            tc.tile_pool(name="in5", bufs=1) as p5, \
            tc.tile_pool(name="out5", bufs=1) as o5, \
            tc.tile_pool(name="s7", bufs=1) as ps7, \
            tc.tile_pool(name="s6", bufs=1) as ps6:
        for s in [s for _ in range(reps) for s in range(s_per_core)]:
            X7 = x7.ap()[s].rearrange("(p q) f -> p (q f)", p=128)  # (128,fl7)
            X6 = x6.ap()[s].rearrange("(p q) f -> p (q f)", p=128)
            X5 = x5.ap()[s].rearrange("(p q) f -> p (q f)", p=128)
            O5 = out.ap()[s, 0:n5].rearrange("(p q) f -> p (q f)", p=128)
            O6 = out.ap()[s, n5:n5 + n6].rearrange("(p q) f -> p (q f)", p=128)
            O7 = out.ap()[s, n5 + n6:].rearrange("(p q) f -> p (q f)", p=128)

            S7 = ps7.tile([128, fl6], _DT)  # group sums of x7 (pre-scale)
            S6 = ps6.tile([128, fl5], _DT)  # group sums of x6 (pre-scale)

            # ---- zoom 7: S7 = gsum4(x7); out7 = x7 - 0.25*rep4(S7) ----
            for j in range(fl7 // ch7):
                t = p7.tile([128, ch7], _DT)
                dma_in(t[:], X7[:, j * ch7:(j + 1) * ch7])
                o = o7.tile([128, ch7], _DT)
                g = ch7 // 128  # parents per partition in this chunk
                t4 = t.rearrange("p (g c f) -> p g c f", c=4, f=F)
                o4 = o.rearrange("p (g c f) -> p g c f", c=4, f=F)
                s3 = S7[:, j * (ch7 // 4):(j + 1) * (ch7 // 4)] \
                    .rearrange("p (g f) -> p g f", f=F)
                nc.vector.tensor_add(s3, t4[:, :, 0], t4[:, :, 1])
                nc.vector.tensor_add(s3, s3, t4[:, :, 2])
                nc.vector.tensor_add(s3, s3, t4[:, :, 3])
                for c in range(4):
                    nc.vector.scalar_tensor_tensor(
                        o4[:, :, c], s3, -0.25, t4[:, :, c],
                        op0=AluOpType.mult, op1=AluOpType.add)
                dma_out(O7[:, j * ch7:(j + 1) * ch7], o[:])

            # ---- zoom 6: S6 = gsum4(x6); out6 = x6 + 0.25*S7 - 0.25*rep4(S6)
            for j in range(fl6 // ch6):
                t = p6.tile([128, ch6], _DT)
                dma_in(t[:], X6[:, j * ch6:(j + 1) * ch6])
                o = o6.tile([128, ch6], _DT)
                g = ch6 // 128
                t4 = t.rearrange("p (g c f) -> p g c f", c=4, f=F)
                o4 = o.rearrange("p (g c f) -> p g c f", c=4, f=F)
                s3 = S6[:, j * (ch6 // 4):(j + 1) * (ch6 // 4)] \
                    .rearrange("p (g f) -> p g f", f=F)
                nc.vector.tensor_add(s3, t4[:, :, 0], t4[:, :, 1])
                nc.vector.tensor_add(s3, s3, t4[:, :, 2])
                nc.vector.tensor_add(s3, s3, t4[:, :, 3])
                # o = x6 + 0.25*S7 (aligned rows)
                nc.vector.scalar_tensor_tensor(
                    o[:], S7[:, j * ch6:(j + 1) * ch6], 0.25, t[:],
                    op0=AluOpType.mult, op1=AluOpType.add)
                for c in range(4):
                    nc.vector.scalar_tensor_tensor(
                        o4[:, :, c], s3, -0.25, o4[:, :, c],
                        op0=AluOpType.mult, op1=AluOpType.add)
                dma_out(O6[:, j * ch6:(j + 1) * ch6], o[:])

            # ---- zoom 5: out5 = x5 + 0.25*S6 ----
            t = p5.tile([128, fl5], _DT)
            dma_in(t[:], X5[:])
            o = o5.tile([128, fl5], _DT)
            nc.vector.scalar_tensor_tensor(
                o[:], S6[:], 0.25, t[:], op0=AluOpType.mult, op1=AluOpType.add)
            dma_out(O5[:], o[:])
    return _legalize_waits(nc)


def build_nc2(s_per_core=S_PER_CORE, fl5=FL5, ch6=1536, reps=1,
              bufs7=2, bufs6=3, bufss7=4, hwdge=True, legalize=True):
    """Stage-interleaved build: each stage covers ch6 floats/partition of the
    x6 view and the matched 4*ch6 span of the x7 view, so input and output
    DMA traffic from both zoom levels coexists throughout the run and the
    vector engine's per-chunk latency hides under the other stages' DMA.
    Loads issue on the sync HWDGE queue, stores on the scalar HWDGE queue.
    """
    fl6, fl7 = 4 * fl5, 16 * fl5
    ch7 = 4 * ch6
    assert fl6 % ch6 == 0 and ch6 % 128 == 0
    n5, n6, n7 = fl5 * 128 // F, fl6 * 128 // F, fl7 * 128 // F
    n_stages = fl6 // ch6

    nc = bass.Bass("TRN2", target_bir_lowering=False, debug=False,
                   enable_asserts=False)
    dma_in = (lambda *a: nc.sync.dma_start(*a)) if hwdge else \
        (lambda *a: nc.gpsimd.dma_start(*a))
    dma_out = (lambda *a: nc.scalar.dma_start(*a)) if hwdge else \
        (lambda *a: nc.gpsimd.dma_start(*a))
    x5 = nc.dram_tensor("x5", [s_per_core, n5, F], _DT, kind="ExternalInput")
    x6 = nc.dram_tensor("x6", [s_per_core, n6, F], _DT, kind="ExternalInput")
    x7 = nc.dram_tensor("x7", [s_per_core, n7, F], _DT, kind="ExternalInput")
    out = nc.dram_tensor("out", [s_per_core, n5 + n6 + n7, F], _DT,
                         kind="ExternalOutput")

    with tile.TileContext(nc) as tc, \
            tc.tile_pool(name="in7", bufs=bufs7) as p7, \
            tc.tile_pool(name="out7", bufs=bufs7) as o7, \
            tc.tile_pool(name="in6", bufs=bufs6) as p6, \
            tc.tile_pool(name="out6", bufs=bufs6) as o6, \
            tc.tile_pool(name="in5", bufs=1) as p5, \
            tc.tile_pool(name="out5", bufs=1) as o5, \
            tc.tile_pool(name="s7", bufs=bufss7) as ps7, \
            tc.tile_pool(name="s6", bufs=1) as ps6:
        for s in [s for _ in range(reps) for s in range(s_per_core)]:
            X7 = x7.ap()[s].rearrange("(p q) f -> p (q f)", p=128)  # (128,fl7)
            X6 = x6.ap()[s].rearrange("(p q) f -> p (q f)", p=128)
            X5 = x5.ap()[s].rearrange("(p q) f -> p (q f)", p=128)
            O5 = out.ap()[s, 0:n5].rearrange("(p q) f -> p (q f)", p=128)
            O6 = out.ap()[s, n5:n5 + n6].rearrange("(p q) f -> p (q f)", p=128)
            O7 = out.ap()[s, n5 + n6:].rearrange("(p q) f -> p (q f)", p=128)

            S6 = ps6.tile([128, fl5], _DT)  # group sums of x6 (pre-scale)

            for k in range(n_stages):
                # ---- x7 part: S7k = gsum4(x7 span); out7 = x7 - 0.25*rep4 --
                t7 = p7.tile([128, ch7], _DT)
                dma_in(t7[:], X7[:, k * ch7:(k + 1) * ch7])
                w7 = o7.tile([128, ch7], _DT)
                S7k = ps7.tile([128, ch6], _DT)
                t4 = t7.rearrange("p (g c f) -> p g c f", c=4, f=F)
                w4 = w7.rearrange("p (g c f) -> p g c f", c=4, f=F)
                s3 = S7k.rearrange("p (g f) -> p g f", f=F)
                nc.vector.tensor_add(s3, t4[:, :, 0], t4[:, :, 1])
                nc.vector.tensor_add(s3, s3, t4[:, :, 2])
                nc.vector.tensor_add(s3, s3, t4[:, :, 3])
                for c in range(4):
                    nc.vector.scalar_tensor_tensor(
                        w4[:, :, c], s3, -0.25, t4[:, :, c],
                        op0=AluOpType.mult, op1=AluOpType.add)
                dma_out(O7[:, k * ch7:(k + 1) * ch7], w7[:])

                # ---- x6 part: S6 span; out6 = x6 + 0.25*S7k - 0.25*rep4 ----
                t6 = p6.tile([128, ch6], _DT)
                dma_in(t6[:], X6[:, k * ch6:(k + 1) * ch6])
                w6 = o6.tile([128, ch6], _DT)
                g6 = ch6 // 4
                t64 = t6.rearrange("p (g c f) -> p g c f", c=4, f=F)
                w64 = w6.rearrange("p (g c f) -> p g c f", c=4, f=F)
                s63 = S6[:, k * g6:(k + 1) * g6].rearrange(
                    "p (g f) -> p g f", f=F)
                nc.vector.tensor_add(s63, t64[:, :, 0], t64[:, :, 1])
                nc.vector.tensor_add(s63, s63, t64[:, :, 2])
                nc.vector.tensor_add(s63, s63, t64[:, :, 3])
                nc.vector.scalar_tensor_tensor(
                    w6[:], S7k[:], 0.25, t6[:],
                    op0=AluOpType.mult, op1=AluOpType.add)
                for c in range(4):
                    nc.vector.scalar_tensor_tensor(
                        w64[:, :, c], s63, -0.25, w64[:, :, c],
                        op0=AluOpType.mult, op1=AluOpType.add)
                dma_out(O6[:, k * ch6:(k + 1) * ch6], w6[:])

            # ---- zoom 5: out5 = x5 + 0.25*S6 ----
            t5 = p5.tile([128, fl5], _DT)
            dma_in(t5[:], X5[:])
            w5 = o5.tile([128, fl5], _DT)
            nc.vector.scalar_tensor_tensor(
                w5[:], S6[:], 0.25, t5[:],
                op0=AluOpType.mult, op1=AluOpType.add)
            dma_out(O5[:], w5[:])
    return _legalize_waits(nc) if legalize else nc


_NC_CACHE = {}


def _get_nc():
    if "nc" not in _NC_CACHE:
        _NC_CACHE["nc"] = build_nc2()
    return _NC_CACHE["nc"]


def kernel(x5: np.ndarray, x6: np.ndarray, x7: np.ndarray) -> np.ndarray:
    x5f = np.ascontiguousarray(x5, dtype=np.float32).reshape(SLICES, N5, F)
    x6f = np.ascontiguousarray(x6, dtype=np.float32).reshape(SLICES, N6, F)
    x7f = np.ascontiguousarray(x7, dtype=np.float32).reshape(SLICES, N7, F)

    in_maps = []
    for c in range(N_CORES):
        lo, hi = c * S_PER_CORE, (c + 1) * S_PER_CORE
        in_maps.append({
            "x5": np.ascontiguousarray(x5f[lo:hi]),
            "x6": np.ascontiguousarray(x6f[lo:hi]),
            "x7": np.ascontiguousarray(x7f[lo:hi]),
        })

    nc = _get_nc()
    res = run_bass_kernel_spmd(nc, in_maps, core_ids=list(range(N_CORES)))
    outs = np.concatenate([res.results[c]["out"] for c in range(N_CORES)],
                          axis=0)  # (16, NOUT, F)
    return outs.reshape(B, V, T, NOUT, 1, F)



# revision 13
# speedup vs baseline: 1.0020x; 1.0020x over previous
"""HEALPix conservative-layer (segment_reduce) Bass kernel for TRN2.

Problem (hardcoded):
  x5: (2,2,4,12288,1,32) f32, x6: (2,2,4,49152,1,32), x7: (2,2,4,196608,1,32)
  out5 = x5 + gmean4(x6)
  out6 = x6 - rep4(gmean4(x6)) + gmean4(x7)
  out7 = x7 - rep4(gmean4(x7))
  out = concat([out5, out6, out7], axis=3)   # (2,2,4,258048,1,32)

Sharding: flatten (b,v,t) -> 16 slices; 8 cores x 2 slices each. Everything is
local to a core.

Layout: one slice (N, 32) is contiguous in DRAM and a parent's 4 children are
128 contiguous floats, so view each slice as (128 partitions, N*32/128) with
each partition a contiguous DRAM block. Parent group-reduction is then along
the free dim and the parent-sum tile S (in the same layout) aligns elementwise
with the next-coarser level's view of the same partition.

Schedule (build_nc2, the one kernel() uses): stage-interleaved streaming.
Each stage covers a matched span (4*ch6 of the x7 view + ch6 of the x6 view),
so input and output DMA traffic from both zoom levels coexists throughout the
run instead of phase-serial x7->x6->x5; the vector-engine latency of any one
chunk hides under the other stages' DMA. Loads issue on the sync HWDGE queue
and stores on the scalar HWDGE queue so next-stage loads never queue behind
dependency-stalled stores. Cost model: 370.9us/core = DMA floor (132MB at
360GB/s = 367.0us) + fixed TileContext preamble/teardown; zero DMA idle
in between. build_nc (phase-serial SWDGE baseline) kept for comparison:
385.4us.
"""

import numpy as np

try:
    import concourse.bass as bass
except ImportError:  # pragma: no cover - fallback for odd sys.path setups
    import sys

    sys.path.insert(0, "/opt/trn_rl_repo")
    import concourse.bass as bass

import concourse.mybir as mybir
import concourse.tile as tile
from concourse.bass_utils import run_bass_kernel_spmd
from concourse.mybir import AluOpType

F = 32
B, V, T = 2, 2, 4
N5, N6, N7 = 12 * 4**5, 12 * 4**6, 12 * 4**7
N_CORES = 8
SLICES = B * V * T  # 16
S_PER_CORE = SLICES // N_CORES  # 2
NOUT = N5 + N6 + N7

# floats per partition in the (128, .) view of one slice
FL5 = N5 * F // 128  # 3072
FL6 = N6 * F // 128  # 12288
FL7 = N7 * F // 128  # 49152
# streaming chunk sizes (floats per partition); must be multiples of 128
CH7 = 3072
CH6 = 2048

_DT = mybir.dt.float32


def _legalize_waits(nc):
    """Split multi-sem-wait instructions: walrus codegen packs at most one
    sync wait into a TPB instruction, so move excess waits onto NoOps inserted
    immediately before (same engine => same in-order semantics)."""
    import copy as _copy

    from bass_rust import SyncInfo

    tmpl = bass.Bass("TRN2").vector.nop().ins
    n = 0
    for fn in nc.m.functions:
        for blk in fn.blocks:
            out = []
            changed = False
            for inst in blk.instructions:
                si = inst.sync_info
                if (si is not None and len(si.on_wait) > 1
                        and all(w.wait_mode == "sem-ge-imm"
                                for w in si.on_wait)):
                    waits = list(si.on_wait)
                    for w in waits[:-1]:
                        nop = _copy.copy(tmpl)
                        nop.name = f"WN-{n}"
                        n += 1
                        nop.engine = inst.engine
                        nop.sync_info = SyncInfo(on_wait=[w], on_update=[])
                        out.append(nop)
                    inst.sync_info = SyncInfo(on_wait=[waits[-1]],
                                              on_update=list(si.on_update))
                    changed = True
                out.append(inst)
            if changed:
                blk.instructions = out
    return nc


def build_nc(s_per_core=S_PER_CORE, fl5=FL5, ch7=CH7, ch6=CH6, reps=1,
             hwdge=False, bufs7=3, bufso7=2):
    """Build the per-core Bass module. All sizes in per-partition floats."""
    fl6, fl7 = 4 * fl5, 16 * fl5
    assert fl6 % 128 == 0 and fl7 % 128 == 0
    assert fl7 % ch7 == 0 and ch7 % 128 == 0
    assert fl6 % ch6 == 0 and ch6 % 128 == 0
    n5, n6, n7 = fl5 * 128 // F, fl6 * 128 // F, fl7 * 128 // F

    nc = bass.Bass("TRN2", target_bir_lowering=False, debug=False,
                   enable_asserts=False)
    dma_in = (lambda *a: nc.sync.dma_start(*a)) if hwdge else \
        (lambda *a: nc.gpsimd.dma_start(*a))
    dma_out = (lambda *a: nc.scalar.dma_start(*a)) if hwdge else \
        (lambda *a: nc.gpsimd.dma_start(*a))
    x5 = nc.dram_tensor("x5", [s_per_core, n5, F], _DT, kind="ExternalInput")
    x6 = nc.dram_tensor("x6", [s_per_core, n6, F], _DT, kind="ExternalInput")
    x7 = nc.dram_tensor("x7", [s_per_core, n7, F], _DT, kind="ExternalInput")
    out = nc.dram_tensor("out", [s_per_core, n5 + n6 + n7, F], _DT,
                         kind="ExternalOutput")

    with tile.TileContext(nc) as tc, \
            tc.tile_pool(name="in7", bufs=bufs7) as p7, \
            tc.tile_pool(name="out7", bufs=bufso7) as o7, \
            tc.tile_pool(name="in6", bufs=2) as p6, \
            tc.tile_pool(name="out6", bufs=2) as o6, \
            tc.tile_pool(name="in5", bufs=1) as p5, \
            tc.tile_pool(name="out5", bufs=1) as o5, \
            tc.tile_pool(name="s7", bufs=1) as ps7, \
            tc.tile_pool(name="s6", bufs=1) as ps6:
        for s in [s for _ in range(reps) for s in range(s_per_core)]:
            X7 = x7.ap()[s].rearrange("(p q) f -> p (q f)", p=128)  # (128,fl7)
            X6 = x6.ap()[s].rearrange("(p q) f -> p (q f)", p=128)
            X5 = x5.ap()[s].rearrange("(p q) f -> p (q f)", p=128)
            O5 = out.ap()[s, 0:n5].rearrange("(p q) f -> p (q f)", p=128)
            O6 = out.ap()[s, n5:n5 + n6].rearrange("(p q) f -> p (q f)", p=128)
            O7 = out.ap()[s, n5 + n6:].rearrange("(p q) f -> p (q f)", p=128)

            S7 = ps7.tile([128, fl6], _DT)  # group sums of x7 (pre-scale)
            S6 = ps6.tile([128, fl5], _DT)  # group sums of x6 (pre-scale)

            # ---- zoom 7: S7 = gsum4(x7); out7 = x7 - 0.25*rep4(S7) ----
            for j in range(fl7 // ch7):
                t = p7.tile([128, ch7], _DT)
                dma_in(t[:], X7[:, j * ch7:(j + 1) * ch7])
                o = o7.tile([128, ch7], _DT)
                g = ch7 // 128  # parents per partition in this chunk
                t4 = t.rearrange("p (g c f) -> p g c f", c=4, f=F)
                o4 = o.rearrange("p (g c f) -> p g c f", c=4, f=F)
                s3 = S7[:, j * (ch7 // 4):(j + 1) * (ch7 // 4)] \
                    .rearrange("p (g f) -> p g f", f=F)
                nc.vector.tensor_add(s3, t4[:, :, 0], t4[:, :, 1])
                nc.vector.tensor_add(s3, s3, t4[:, :, 2])
                nc.vector.tensor_add(s3, s3, t4[:, :, 3])
                for c in range(4):
                    nc.vector.scalar_tensor_tensor(
                        o4[:, :, c], s3, -0.25, t4[:, :, c],
                        op0=AluOpType.mult, op1=AluOpType.add)
                dma_out(O7[:, j * ch7:(j + 1) * ch7], o[:])

            # ---- zoom 6: S6 = gsum4(x6); out6 = x6 + 0.25*S7 - 0.25*rep4(S6)
            for j in range(fl6 // ch6):
                t = p6.tile([128, ch6], _DT)
                dma_in(t[:], X6[:, j * ch6:(j + 1) * ch6])
                o = o6.tile([128, ch6], _DT)
                g = ch6 // 128
                t4 = t.rearrange("p (g c f) -> p g c f", c=4, f=F)
                o4 = o.rearrange("p (g c f) -> p g c f", c=4, f=F)
                s3 = S6[:, j * (ch6 // 4):(j + 1) * (ch6 // 4)] \
                    .rearrange("p (g f) -> p g f", f=F)
                nc.vector.tensor_add(s3, t4[:, :, 0], t4[:, :, 1])
                nc.vector.tensor_add(s3, s3, t4[:, :, 2])
                nc.vector.tensor_add(s3, s3, t4[:, :, 3])
                # o = x6 + 0.25*S7 (aligned rows)
                nc.vector.scalar_tensor_tensor(
                    o[:], S7[:, j * ch6:(j + 1) * ch6], 0.25, t[:],
                    op0=AluOpType.mult, op1=AluOpType.add)
                for c in range(4):
                    nc.vector.scalar_tensor_tensor(
                        o4[:, :, c], s3, -0.25, o4[:, :, c],
                        op0=AluOpType.mult, op1=AluOpType.add)
                dma_out(O6[:, j * ch6:(j + 1) * ch6], o[:])

            # ---- zoom 5: out5 = x5 + 0.25*S6 ----
            t = p5.tile([128, fl5], _DT)
            dma_in(t[:], X5[:])
            o = o5.tile([128, fl5], _DT)
            nc.vector.scalar_tensor_tensor(
                o[:], S6[:], 0.25, t[:], op0=AluOpType.mult, op1=AluOpType.add)
            dma_out(O5[:], o[:])
    return _legalize_waits(nc)


def _hoist_first_dmas(nc, n_hoist):
    """Move the first wait-free SP dma_starts from the body block to just
    before SP's start-barrier EventSemaphore in the preamble block, so their
    descriptor generation overlaps the global engine barrier. Safe because
    they read external DRAM, write SBUF addresses nothing in the preamble
    touches, and only increment their own completion semaphores."""
    fn = nc.m.functions[0]
    pre, body = fn.blocks[0], fn.blocks[1]
    sp = mybir.EngineType.SP
    moved = []
    for inst in body.instructions:
        if len(moved) >= n_hoist:
            break
        if inst.engine != sp:
            continue
        if type(inst).__name__ != "InstDMACopy":
            continue
        si = inst.sync_info
        if si is not None and si.on_wait:
            break  # stop at the first waiting DMA to preserve issue order
        moved.append(inst)
    if not moved:
        return nc
    body.instructions = [i for i in body.instructions if i not in moved]
    idx = next(i for i, inst in enumerate(pre.instructions)
               if inst.engine == sp
               and type(inst).__name__ == "InstEventSemaphore")
    pre.instructions = (pre.instructions[:idx] + moved
                        + pre.instructions[idx:])
    return nc


def build_nc2(s_per_core=S_PER_CORE, fl5=FL5, ch6=1536, reps=1,
              bufs7=2, bufs6=3, bufss7=4, hwdge=True, legalize=True, m6=1,
              hoist=1):
    """Stage-interleaved build: each stage covers ch6 floats/partition of the
    x6 view and the matched 4*ch6 span of the x7 view, so input and output
    DMA traffic from both zoom levels coexists throughout the run and the
    vector engine's per-chunk latency hides under the other stages' DMA.
    Loads issue on the sync HWDGE queue, stores on the scalar HWDGE queue.
    m6: stages per x6 load/store DMA (merging keeps every DMA >=1.5MB on HW).
    """
    fl6, fl7 = 4 * fl5, 16 * fl5
    ch7 = 4 * ch6
    assert fl6 % ch6 == 0 and ch6 % 128 == 0
    n5, n6, n7 = fl5 * 128 // F, fl6 * 128 // F, fl7 * 128 // F
    n_stages = fl6 // ch6
    assert n_stages % m6 == 0

    nc = bass.Bass("TRN2", target_bir_lowering=False, debug=False,
                   enable_asserts=False)
    dma_in = (lambda *a: nc.sync.dma_start(*a)) if hwdge else \
        (lambda *a: nc.gpsimd.dma_start(*a))
    dma_out = (lambda *a: nc.scalar.dma_start(*a)) if hwdge else \
        (lambda *a: nc.gpsimd.dma_start(*a))
    x5 = nc.dram_tensor("x5", [s_per_core, n5, F], _DT, kind="ExternalInput")
    x6 = nc.dram_tensor("x6", [s_per_core, n6, F], _DT, kind="ExternalInput")
    x7 = nc.dram_tensor("x7", [s_per_core, n7, F], _DT, kind="ExternalInput")
    out = nc.dram_tensor("out", [s_per_core, n5 + n6 + n7, F], _DT,
                         kind="ExternalOutput")

    with tile.TileContext(nc) as tc, \
            tc.tile_pool(name="in7", bufs=bufs7) as p7, \
            tc.tile_pool(name="out7", bufs=bufs7) as o7, \
            tc.tile_pool(name="in6", bufs=bufs6) as p6, \
            tc.tile_pool(name="out6", bufs=bufs6) as o6, \
            tc.tile_pool(name="in5", bufs=1) as p5, \
            tc.tile_pool(name="out5", bufs=1) as o5, \
            tc.tile_pool(name="s7", bufs=bufss7) as ps7, \
            tc.tile_pool(name="s6", bufs=1) as ps6:
        for s in [s for _ in range(reps) for s in range(s_per_core)]:
            X7 = x7.ap()[s].rearrange("(p q) f -> p (q f)", p=128)  # (128,fl7)
            X6 = x6.ap()[s].rearrange("(p q) f -> p (q f)", p=128)
            X5 = x5.ap()[s].rearrange("(p q) f -> p (q f)", p=128)
            O5 = out.ap()[s, 0:n5].rearrange("(p q) f -> p (q f)", p=128)
            O6 = out.ap()[s, n5:n5 + n6].rearrange("(p q) f -> p (q f)", p=128)
            O7 = out.ap()[s, n5 + n6:].rearrange("(p q) f -> p (q f)", p=128)

            S6 = ps6.tile([128, fl5], _DT)  # group sums of x6 (pre-scale)

            t6b = w6b = None
            for k in range(n_stages):
                # ---- x7 part: S7k = gsum4(x7 span); out7 = x7 - 0.25*rep4 --
                t7 = p7.tile([128, ch7], _DT)
                dma_in(t7[:], X7[:, k * ch7:(k + 1) * ch7])
                if k % m6 == 0:
                    # x6 load covers the next m6 stages (issued after the in7
                    # load so the in-queue stays in steady-state order)
                    t6b = p6.tile([128, m6 * ch6], _DT)
                    dma_in(t6b[:], X6[:, k * ch6:(k + m6) * ch6])
                    w6b = o6.tile([128, m6 * ch6], _DT)
                w7 = o7.tile([128, ch7], _DT)
                S7k = ps7.tile([128, ch6], _DT)
                t4 = t7.rearrange("p (g c f) -> p g c f", c=4, f=F)
                w4 = w7.rearrange("p (g c f) -> p g c f", c=4, f=F)
                s3 = S7k.rearrange("p (g f) -> p g f", f=F)
                nc.vector.tensor_add(s3, t4[:, :, 0], t4[:, :, 1])
                nc.vector.tensor_add(s3, s3, t4[:, :, 2])
                nc.vector.tensor_add(s3, s3, t4[:, :, 3])
                for c in range(4):
                    nc.vector.scalar_tensor_tensor(
                        w4[:, :, c], s3, -0.25, t4[:, :, c],
                        op0=AluOpType.mult, op1=AluOpType.add)
                dma_out(O7[:, k * ch7:(k + 1) * ch7], w7[:])

                # ---- x6 part: S6 span; out6 = x6 + 0.25*S7k - 0.25*rep4 ----
                j = k % m6
                t6 = t6b[:, j * ch6:(j + 1) * ch6]
                w6 = w6b[:, j * ch6:(j + 1) * ch6]
                g6 = ch6 // 4
                t64 = t6.rearrange("p (g c f) -> p g c f", c=4, f=F)
                w64 = w6.rearrange("p (g c f) -> p g c f", c=4, f=F)
                s63 = S6[:, k * g6:(k + 1) * g6].rearrange(
                    "p (g f) -> p g f", f=F)
                nc.vector.tensor_add(s63, t64[:, :, 0], t64[:, :, 1])
                nc.vector.tensor_add(s63, s63, t64[:, :, 2])
                nc.vector.tensor_add(s63, s63, t64[:, :, 3])
                nc.vector.scalar_tensor_tensor(
                    w6, S7k[:], 0.25, t6,
                    op0=AluOpType.mult, op1=AluOpType.add)
                for c in range(4):
                    nc.vector.scalar_tensor_tensor(
                        w64[:, :, c], s63, -0.25, w64[:, :, c],
                        op0=AluOpType.mult, op1=AluOpType.add)
                if j == m6 - 1:
                    dma_out(O6[:, (k + 1 - m6) * ch6:(k + 1) * ch6], w6b[:])

            # ---- zoom 5: out5 = x5 + 0.25*S6 ----
            t5 = p5.tile([128, fl5], _DT)
            dma_in(t5[:], X5[:])
            w5 = o5.tile([128, fl5], _DT)
            nc.vector.scalar_tensor_tensor(
                w5[:], S6[:], 0.25, t5[:],
                op0=AluOpType.mult, op1=AluOpType.add)
            dma_out(O5[:], w5[:])
    if hoist:
        _hoist_first_dmas(nc, hoist)
    return _legalize_waits(nc) if legalize else nc


_NC_CACHE = {}


def _get_nc():
    if "nc" not in _NC_CACHE:
        _NC_CACHE["nc"] = build_nc2()
    return _NC_CACHE["nc"]


def kernel(x5: np.ndarray, x6: np.ndarray, x7: np.ndarray) -> np.ndarray:
    x5f = np.ascontiguousarray(x5, dtype=np.float32).reshape(SLICES, N5, F)
    x6f = np.ascontiguousarray(x6, dtype=np.float32).reshape(SLICES, N6, F)
    x7f = np.ascontiguousarray(x7, dtype=np.float32).reshape(SLICES, N7, F)

    in_maps = []
    for c in range(N_CORES):
        lo, hi = c * S_PER_CORE, (c + 1) * S_PER_CORE
        in_maps.append({
            "x5": np.ascontiguousarray(x5f[lo:hi]),
            "x6": np.ascontiguousarray(x6f[lo:hi]),
            "x7": np.ascontiguousarray(x7f[lo:hi]),
        })

    nc = _get_nc()
    res = run_bass_kernel_spmd(nc, in_maps, core_ids=list(range(N_CORES)))
    outs = np.concatenate([res.results[c]["out"] for c in range(N_CORES)],
                          axis=0)  # (16, NOUT, F)
    return outs.reshape(B, V, T, NOUT, 1, F)

